# revision 5
# baseline (speedup 1.0000x reference)
"""Trainium2 kernel for nn_CachedReadoutModel (PCA -> MLP -> species shift -> segment sum).

Strategy (8 NeuronCores, data-parallel over atoms):
  host:  fold PCA into layer 1 (W_eff = (W1 @ pca_components).T, b_eff = b1 - W_eff.T mean),
         slice 1M atoms into 8 ranges, pad each to 128*T, stage x transposed
         (feature-major) in bf16, batch_map/node_attrs tile-transposed.
  core:  h = silu(W_eff.T x + b_eff); mlp = h . w2
         tot = mlp + (shifts + b2)[argmax(node_attrs)]   (exact first-index argmax)
         E[bm>>7, bm&127] += tot  via one-hot matmuls accumulated in PSUM:
            E += oh_hi^T @ (oh_lo * tot_hi) + oh_hi^T @ (oh_lo * tot_lo)
         with tot = tot_hi + tot_lo split so each part is fp16-exact.
  host:  delta = sum over cores of E; final = base_energy + delta.
"""

import os
import sys

for _p in ("/opt/trn_rl_repo", "/root/.axon_site/_ro/trn_rl_repo"):
    if os.path.isdir(_p) and _p not in sys.path:
        sys.path.insert(0, _p)

from contextlib import ExitStack

import numpy as np
import ml_dtypes

import concourse.bass as bass
import concourse.tile as tile
from concourse import bacc, mybir
from concourse._compat import with_exitstack
from concourse.bass_utils import run_bass_kernel_spmd

dt = mybir.dt
Alu = mybir.AluOpType
Act = mybir.ActivationFunctionType

N_ATOMS = 1_000_000
N_GRAPHS = 16384
NS = 10
N_CORES = 8
T = 992  # tiles of 128 atoms per core; A = 126976 >= ceil(1e6/8)
A = 128 * T
PAD_BM = 1 << 14  # hi = 128 -> one-hot row all zero -> padding atoms contribute nothing
TRUNC_MASK = int(~np.int32(0x1FFF))  # keep 10 explicit mantissa bits -> fp16-exact

_PROGRAM_CACHE = {}


@with_exitstack
def _emit_body(ctx: ExitStack, tc, T, ins, e_out, with_b2=True, cpath_chunks=4, silu_mode="act"):
    nc = tc.nc
    assert T % 16 == 0
    n_super = T // 16

    const = ctx.enter_context(tc.tile_pool(name="const", bufs=1))
    cpath = ctx.enter_context(tc.tile_pool(name="cpath", bufs=2))
    xpool = ctx.enter_context(tc.tile_pool(name="xpool", bufs=3))
    work = ctx.enter_context(tc.tile_pool(name="work", bufs=3))
    oh = ctx.enter_context(tc.tile_pool(name="oh", bufs=4))
    hps = ctx.enter_context(tc.tile_pool(name="hps", bufs=2, space="PSUM"))
    mps = ctx.enter_context(tc.tile_pool(name="mps", bufs=2, space="PSUM"))
    eps = ctx.enter_context(tc.tile_pool(name="eps", bufs=1, space="PSUM"))

    def load_const(name, shape, dtype):
        t = const.tile(shape, dtype, tag=name)
        nc.sync.dma_start(t[:], ins[name])
        return t

    wa = load_const("wa", [128, 128], dt.bfloat16)
    wb = load_const("wb", [64, 128], dt.bfloat16)
    w2c = load_const("w2c", [128, 1], dt.float16)
    beff = load_const("beff", [128, 1], dt.float32)
    shiftsb = load_const("shiftsb", [128, NS], dt.float32)
    wpow = load_const("wpow", [128, NS], dt.float32)
    iota10n = load_const("iota10n", [128, NS], dt.float32)
    iotaf = load_const("iotaf", [128, 128], dt.float16)
    bmt = load_const("bmt", [128, T], dt.int32)

    hi_i = const.tile([128, T], dt.int32)
    nc.vector.tensor_scalar(hi_i[:], bmt[:], 7, None, Alu.logical_shift_right)
    hi_t = const.tile([128, T], dt.float32)
    nc.vector.tensor_copy(hi_t[:], hi_i[:])
    lo_i = const.tile([128, T], dt.int32)
    nc.vector.tensor_scalar(lo_i[:], bmt[:], 127, None, Alu.bitwise_and)
    lo_t = const.tile([128, T], dt.float32)
    nc.vector.tensor_copy(lo_t[:], lo_i[:])

    # --- c table: c[p, t] = (shifts + b2)[argmax_s na[p, t, :]] ---
    c_all = const.tile([128, T], dt.float32)
    assert T % cpath_chunks == 0
    Tc = T // cpath_chunks
    for ci in range(cpath_chunks):
        nat_c = cpath.tile([128, Tc * NS], dt.float32, tag="natc")
        nc.sync.dma_start(nat_c[:], ins["nat"][:, ci * Tc * NS : (ci + 1) * Tc * NS])
        nat3 = nat_c[:].rearrange("p (t s) -> p t s", s=NS)
        mx = cpath.tile([128, Tc], dt.float32, tag="mx")
        nc.vector.tensor_reduce(out=mx[:], in_=nat3, op=Alu.max, axis=mybir.AxisListType.X)
        eq = cpath.tile([128, Tc * NS], dt.float32, tag="eq")
        eq3 = eq[:].rearrange("p (t s) -> p t s", s=NS)
        nc.vector.tensor_tensor(eq3, nat3, mx[:].unsqueeze(-1).broadcast_to([128, Tc, NS]), Alu.is_equal)
        rw = cpath.tile([128, Tc * NS], dt.float32, tag="rw")
        rw3 = rw[:].rearrange("p (t s) -> p t s", s=NS)
        nc.vector.tensor_tensor(rw3, eq3, wpow[:].unsqueeze(1).broadcast_to([128, Tc, NS]), Alu.mult)
        r = cpath.tile([128, Tc], dt.float32, tag="r")
        nc.vector.tensor_reduce(out=r[:], in_=rw3, op=Alu.add, axis=mybir.AxisListType.X)
        # r = sum_s 2^(9-s) [na == max]; exponent of r encodes FIRST argmax index
        em_i = cpath.tile([128, Tc], dt.int32, tag="emi")
        nc.vector.tensor_scalar(em_i[:], r[:].bitcast(dt.int32), 23, None, Alu.logical_shift_right)
        em = cpath.tile([128, Tc], dt.float32, tag="em")
        nc.vector.tensor_scalar(em[:], em_i[:], 136, None, Alu.subtract)
        eq2 = cpath.tile([128, Tc * NS], dt.float32, tag="eq2")
        eq23 = eq2[:].rearrange("p (t s) -> p t s", s=NS)
        nc.vector.tensor_tensor(
            eq23,
            iota10n[:].unsqueeze(1).broadcast_to([128, Tc, NS]),
            em[:].unsqueeze(-1).broadcast_to([128, Tc, NS]),
            Alu.is_equal,
        )
        cw = cpath.tile([128, Tc * NS], dt.float32, tag="cw")
        cw3 = cw[:].rearrange("p (t s) -> p t s", s=NS)
        nc.vector.tensor_tensor(cw3, eq23, shiftsb[:].unsqueeze(1).broadcast_to([128, Tc, NS]), Alu.mult)
        nc.vector.tensor_reduce(out=c_all[:, ci * Tc : (ci + 1) * Tc], in_=cw3, op=Alu.add, axis=mybir.AxisListType.X)

    # --- main loop over superblocks of 2048 atoms (16 tiles) ---
    e_ps = eps.tile([128, 128], dt.float32)
    n_seg_mm = T * (2 if with_b2 else 1)
    seg_i = 0
    for s in range(n_super):
        a0 = s * 2048
        x1 = xpool.tile([128, 2048], dt.bfloat16, tag="x1")
        nc.sync.dma_start(x1[:], ins["xt1"][:, a0 : a0 + 2048])
        x2 = xpool.tile([64, 2048], dt.bfloat16, tag="x2")
        nc.sync.dma_start(x2[:], ins["xt2"][:, a0 : a0 + 2048])
        mlp_ps = mps.tile([128, 16], dt.float32)
        for b in range(4):
            h_ps = hps.tile([128, 512], dt.float32)
            nc.tensor.matmul(h_ps[:], wa[:], x1[:, b * 512 : (b + 1) * 512], start=True, stop=False)
            nc.tensor.matmul(h_ps[:], wb[:], x2[:, b * 512 : (b + 1) * 512], start=False, stop=True)
            silu = work.tile([128, 512], dt.float16, tag="silu")
            if silu_mode == "act":
                nc.scalar.activation(silu[:], h_ps[:], Act.Silu, bias=beff[:], scale=1.0)
            else:
                sg = work.tile([128, 512], dt.float32, tag="sg")
                nc.scalar.activation(sg[:], h_ps[:], Act.Sigmoid, bias=beff[:], scale=1.0)
                hb = work.tile([128, 512], dt.float32, tag="hb")
                nc.scalar.activation(hb[:], h_ps[:], Act.Identity, bias=beff[:], scale=1.0)
                nc.vector.tensor_tensor(silu[:], hb[:], sg[:], Alu.mult)
            for j in range(4):
                nc.tensor.matmul(
                    mlp_ps[:, b * 4 + j : b * 4 + j + 1],
                    silu[:, j * 128 : (j + 1) * 128],
                    w2c[:],
                    start=True,
                    stop=True,
                )
        tot = work.tile([128, 16], dt.float32, tag="tot")
        nc.vector.tensor_tensor(tot[:], mlp_ps[:], c_all[:, s * 16 : (s + 1) * 16], Alu.add)
        tothi = work.tile([128, 16], dt.int32, tag="tothi")
        nc.vector.tensor_scalar(tothi[:], tot[:].bitcast(dt.int32), TRUNC_MASK, None, Alu.bitwise_and)
        totlo = work.tile([128, 16], dt.float32, tag="totlo")
        nc.vector.tensor_tensor(totlo[:], tot[:], tothi[:].bitcast(dt.float32), Alu.subtract)
        for k in range(16):
            t = s * 16 + k
            ohk = oh.tile([128, 128], dt.float16, tag="ohhi")
            nc.gpsimd.tensor_scalar(ohk[:], iotaf[:], hi_t[:, t : t + 1], None, Alu.is_equal)
            b1 = oh.tile([128, 128], dt.float16, tag="b1")
            nc.vector.tensor_scalar(
                b1[:], iotaf[:], lo_t[:, t : t + 1], tothi[:, k : k + 1].bitcast(dt.float32), Alu.is_equal, Alu.mult
            )
            nc.tensor.matmul(e_ps[:], ohk[:], b1[:], start=(seg_i == 0), stop=(seg_i == n_seg_mm - 1), skip_group_check=True)
            seg_i += 1
            if with_b2:
                b2t = oh.tile([128, 128], dt.float16, tag="b2")
                nc.vector.tensor_scalar(b2t[:], iotaf[:], lo_t[:, t : t + 1], totlo[:, k : k + 1], Alu.is_equal, Alu.mult)
                nc.tensor.matmul(e_ps[:], ohk[:], b2t[:], start=False, stop=(seg_i == n_seg_mm - 1), skip_group_check=True)
                seg_i += 1

    e_sb = const.tile([128, 128], dt.float32)
    nc.vector.tensor_copy(e_sb[:], e_ps[:])
    nc.sync.dma_start(e_out, e_sb[:])


def _build_program(T, with_b2=True, cpath_chunks=4, silu_mode="act"):
    A_ = 128 * T
    nc = bacc.Bacc("TRN2", target_bir_lowering=False, debug=False)
    shapes = {
        "xt1": ([128, A_], dt.bfloat16),
        "xt2": ([64, A_], dt.bfloat16),
        "bmt": ([128, T], dt.int32),
        "nat": ([128, T * NS], dt.float32),
        "wa": ([128, 128], dt.bfloat16),
        "wb": ([64, 128], dt.bfloat16),
        "w2c": ([128, 1], dt.float16),
        "beff": ([128, 1], dt.float32),
        "shiftsb": ([128, NS], dt.float32),
        "wpow": ([128, NS], dt.float32),
        "iota10n": ([128, NS], dt.float32),
        "iotaf": ([128, 128], dt.float16),
    }
    ins = {name: nc.declare_dram_parameter(name, list(sh), d, isOutput=False).ap() for name, (sh, d) in shapes.items()}
    e_out = nc.declare_dram_parameter("e_out", [128, 128], dt.float32, isOutput=True).ap()
    with tile.TileContext(nc) as tc:
        _emit_body(tc, T, ins, e_out, with_b2=with_b2, cpath_chunks=cpath_chunks, silu_mode=silu_mode)
    nc.finalize()
    return nc


def _stage_params(pca_mean, pca_components, W1, b1, W2, b2, shifts):
    W_eff = (W1.astype(np.float64) @ pca_components.astype(np.float64)).T  # [192, 128]
    b_eff = b1.astype(np.float64) - W_eff.T @ pca_mean.astype(np.float64)
    W_eff = W_eff.astype(np.float32)
    bf = ml_dtypes.bfloat16
    return {
        "wa": np.ascontiguousarray(W_eff[:128]).astype(bf),
        "wb": np.ascontiguousarray(W_eff[128:]).astype(bf),
        "w2c": np.ascontiguousarray(W2.reshape(128, 1)).astype(np.float16),
        "beff": b_eff.astype(np.float32).reshape(128, 1),
        "shiftsb": np.broadcast_to((shifts + b2[0]).astype(np.float32), (128, NS)).copy(),
        "wpow": np.broadcast_to((2.0 ** (9 - np.arange(NS))).astype(np.float32), (128, NS)).copy(),
        "iota10n": np.broadcast_to((-np.arange(NS)).astype(np.float32), (128, NS)).copy(),
        "iotaf": np.broadcast_to(np.arange(128).astype(np.float16), (128, 128)).copy(),
    }


def _stage_core_inputs(x_c, na_c, bm_c):
    n = x_c.shape[0]
    bf = ml_dtypes.bfloat16
    xt = np.zeros((192, A), dtype=bf)
    xt[:, :n] = np.ascontiguousarray(x_c.T).astype(bf)
    bmt = np.full((A,), PAD_BM, dtype=np.int32)
    bmt[:n] = bm_c.astype(np.int32)
    bmt = np.ascontiguousarray(bmt.reshape(T, 128).T)
    nat = np.zeros((A, NS), dtype=np.float32)
    nat[:n] = na_c
    nat = np.ascontiguousarray(nat.reshape(T, 128, NS).transpose(1, 0, 2).reshape(128, T * NS))
    return {
        "xt1": np.ascontiguousarray(xt[:128]),
        "xt2": np.ascontiguousarray(xt[128:]),
        "bmt": bmt,
        "nat": nat,
    }


def _get_program():
    key = (T, True, "act")
    if key not in _PROGRAM_CACHE:
        _PROGRAM_CACHE[key] = _build_program(T, with_b2=True, silu_mode="act")
    return _PROGRAM_CACHE[key]


def kernel(x, node_attrs, batch_map, base_energy, pca_mean, pca_components, W1, b1, W2, b2, shifts, _trace=False):
    x = np.asarray(x, dtype=np.float32)
    node_attrs = np.asarray(node_attrs, dtype=np.float32)
    batch_map = np.asarray(batch_map)
    base_energy = np.asarray(base_energy, dtype=np.float32)
    params = _stage_params(
        np.asarray(pca_mean, np.float32),
        np.asarray(pca_components, np.float32),
        np.asarray(W1, np.float32),
        np.asarray(b1, np.float32),
        np.asarray(W2, np.float32),
        np.asarray(b2, np.float32),
        np.asarray(shifts, np.float32),
    )

    n = x.shape[0]
    bounds = [min((n + N_CORES - 1) // N_CORES * c, n) for c in range(N_CORES + 1)]
    in_maps = []
    for c in range(N_CORES):
        s, e = bounds[c], bounds[c + 1]
        m = _stage_core_inputs(x[s:e], node_attrs[s:e], batch_map[s:e])
        m.update(params)
        in_maps.append(m)

    nc = _get_program()
    res = run_bass_kernel_spmd(nc, in_maps, list(range(N_CORES)), trace=_trace)
    e_parts = [np.asarray(r["e_out"], dtype=np.float64) for r in res.results]
    delta = np.sum(e_parts, axis=0).reshape(N_GRAPHS).astype(np.float32)
    final = base_energy + delta
    if _trace:
        kernel._last_result = res
    return final, delta


# revision 7
# speedup vs baseline: 4.8257x; 4.8257x over previous
"""Trainium2 kernel for nn_CachedReadoutModel (PCA -> MLP -> species shift -> segment sum).

Strategy (8 NeuronCores, data-parallel over atoms):
  host:  fold PCA into layer 1 (W_eff = (W1 @ pca_components).T, b_eff = b1 - W_eff.T mean);
         slice 1M atoms into 8 ranges; within each range STABLE-SORT atoms by
         batch_map so each 128-atom tile spans <= 32 consecutive graphs; stage x
         transposed (feature-major) in bf16; precompute per-tile segment matrices
         S[t] in [128 atoms, 32 local graphs] (0/1, fp16) from the sorted batch_map.
  core:  h = silu(W_eff.T x + b_eff); mlp = h . w2            (bf16/fp16 matmuls)
         tot = mlp + (shifts + b2)[argmax(node_attrs)]         (exact argmax on DVE)
         split tot = tot_hi + tot_lo (fp16-exact pieces)
         per tile: out[0:32, 2t:2t+2] = S[t]^T @ [tot_hi | tot_lo]   (PE, N=2)
  host:  scatter-add the per-tile per-local-graph partial sums into delta[16384]
         (<= 18k values per core), sum cores, final = base_energy + delta.
"""

import os
import sys

for _p in ("/opt/trn_rl_repo", "/root/.axon_site/_ro/trn_rl_repo"):
    if os.path.isdir(_p) and _p not in sys.path:
        sys.path.insert(0, _p)

from contextlib import ExitStack

import numpy as np
import ml_dtypes

import concourse.bass as bass
import concourse.tile as tile
from concourse import bacc, mybir
from concourse._compat import with_exitstack
from concourse.bass_utils import run_bass_kernel_spmd

dt = mybir.dt
Alu = mybir.AluOpType
Act = mybir.ActivationFunctionType

N_ATOMS = 1_000_000
N_GRAPHS = 16384
NS = 10
N_CORES = 8
T = 992  # tiles of 128 atoms per core; A = 126976 >= ceil(1e6/8)
A = 128 * T
GPT = 32  # default max graphs per 128-atom tile (sorted); host adapts via _pick_gpt
TRUNC_MASK = int(~np.int32(0x1FFF))  # keep 10 explicit mantissa bits -> fp16-exact

_PROGRAM_CACHE = {}


@with_exitstack
def _emit_body(ctx: ExitStack, tc, T, ins, e_out, gpt=GPT, cpath_chunks=4, silu_mode="act"):
    nc = tc.nc
    assert T % 16 == 0
    n_super = T // 16

    const = ctx.enter_context(tc.tile_pool(name="const", bufs=1))
    cpath = ctx.enter_context(tc.tile_pool(name="cpath", bufs=2))
    xpool = ctx.enter_context(tc.tile_pool(name="xpool", bufs=3))
    spool = ctx.enter_context(tc.tile_pool(name="spool", bufs=3))
    work = ctx.enter_context(tc.tile_pool(name="work", bufs=3))
    hps = ctx.enter_context(tc.tile_pool(name="hps", bufs=2, space="PSUM"))
    mps = ctx.enter_context(tc.tile_pool(name="mps", bufs=2, space="PSUM"))
    eps = ctx.enter_context(tc.tile_pool(name="eps", bufs=1, space="PSUM"))

    def load_const(name, shape, dtype):
        t = const.tile(shape, dtype, tag=name)
        nc.sync.dma_start(t[:], ins[name])
        return t

    wa = load_const("wa", [128, 128], dt.bfloat16)
    wb = load_const("wb", [64, 128], dt.bfloat16)
    w2c = load_const("w2c", [128, 1], dt.float16)
    beff = load_const("beff", [128, 1], dt.float32)
    shiftsb = load_const("shiftsb", [128, NS], dt.float32)
    wpow = load_const("wpow", [128, NS], dt.float32)
    iota10n = load_const("iota10n", [128, NS], dt.float32)

    # --- c table: c[p, t] = (shifts + b2)[argmax_s na[p, t, :]] (exact first-index) ---
    c_all = const.tile([128, T], dt.float32)
    assert T % cpath_chunks == 0
    Tc = T // cpath_chunks
    for ci in range(cpath_chunks):
        nat_c = cpath.tile([128, Tc * NS], dt.float32, tag="natc")
        nc.sync.dma_start(nat_c[:], ins["nat"][:, ci * Tc * NS : (ci + 1) * Tc * NS])
        nat3 = nat_c[:].rearrange("p (t s) -> p t s", s=NS)
        mx = cpath.tile([128, Tc], dt.float32, tag="mx")
        nc.vector.tensor_reduce(out=mx[:], in_=nat3, op=Alu.max, axis=mybir.AxisListType.X)
        eq = cpath.tile([128, Tc * NS], dt.float32, tag="eq")
        eq3 = eq[:].rearrange("p (t s) -> p t s", s=NS)
        nc.vector.tensor_tensor(eq3, nat3, mx[:].unsqueeze(-1).broadcast_to([128, Tc, NS]), Alu.is_equal)
        rw = cpath.tile([128, Tc * NS], dt.float32, tag="rw")
        rw3 = rw[:].rearrange("p (t s) -> p t s", s=NS)
        nc.vector.tensor_tensor(rw3, eq3, wpow[:].unsqueeze(1).broadcast_to([128, Tc, NS]), Alu.mult)
        r = cpath.tile([128, Tc], dt.float32, tag="r")
        nc.vector.tensor_reduce(out=r[:], in_=rw3, op=Alu.add, axis=mybir.AxisListType.X)
        em_i = cpath.tile([128, Tc], dt.int32, tag="emi")
        nc.vector.tensor_scalar(em_i[:], r[:].bitcast(dt.int32), 23, None, Alu.logical_shift_right)
        em = cpath.tile([128, Tc], dt.float32, tag="em")
        nc.vector.tensor_scalar(em[:], em_i[:], 136, None, Alu.subtract)
        eq2 = cpath.tile([128, Tc * NS], dt.float32, tag="eq2")
        eq23 = eq2[:].rearrange("p (t s) -> p t s", s=NS)
        nc.vector.tensor_tensor(
            eq23,
            iota10n[:].unsqueeze(1).broadcast_to([128, Tc, NS]),
            em[:].unsqueeze(-1).broadcast_to([128, Tc, NS]),
            Alu.is_equal,
        )
        cw = cpath.tile([128, Tc * NS], dt.float32, tag="cw")
        cw3 = cw[:].rearrange("p (t s) -> p t s", s=NS)
        nc.vector.tensor_tensor(cw3, eq23, shiftsb[:].unsqueeze(1).broadcast_to([128, Tc, NS]), Alu.mult)
        nc.vector.tensor_reduce(out=c_all[:, ci * Tc : (ci + 1) * Tc], in_=cw3, op=Alu.add, axis=mybir.AxisListType.X)

    # --- main loop over superblocks of 2048 atoms (16 tiles) ---
    e_ps = eps.tile([128, 2 * T], dt.float32)
    for s in range(n_super):
        a0 = s * 2048
        x1 = xpool.tile([128, 2048], dt.bfloat16, tag="x1")
        nc.sync.dma_start(x1[:], ins["xt1"][:, a0 : a0 + 2048])
        x2 = xpool.tile([64, 2048], dt.bfloat16, tag="x2")
        nc.sync.dma_start(x2[:], ins["xt2"][:, a0 : a0 + 2048])
        st = spool.tile([128, 16 * gpt], dt.float16, tag="st")
        nc.sync.dma_start(st[:], ins["seg"][:, s * 16 * gpt : (s + 1) * 16 * gpt])
        mlp_ps = mps.tile([128, 16], dt.float32)
        for b in range(4):
            h_ps = hps.tile([128, 512], dt.float32)
            nc.tensor.matmul(h_ps[:], wa[:], x1[:, b * 512 : (b + 1) * 512], start=True, stop=False)
            nc.tensor.matmul(h_ps[:], wb[:], x2[:, b * 512 : (b + 1) * 512], start=False, stop=True)
            silu = work.tile([128, 512], dt.float16, tag="silu")
            if silu_mode == "act":
                nc.scalar.activation(silu[:], h_ps[:], Act.Silu, bias=beff[:], scale=1.0)
            else:
                sg = work.tile([128, 512], dt.float32, tag="sg")
                nc.scalar.activation(sg[:], h_ps[:], Act.Sigmoid, bias=beff[:], scale=1.0)
                hb = work.tile([128, 512], dt.float32, tag="hb")
                nc.scalar.activation(hb[:], h_ps[:], Act.Identity, bias=beff[:], scale=1.0)
                nc.vector.tensor_tensor(silu[:], hb[:], sg[:], Alu.mult)
            for j in range(4):
                nc.tensor.matmul(
                    mlp_ps[:, b * 4 + j : b * 4 + j + 1],
                    silu[:, j * 128 : (j + 1) * 128],
                    w2c[:],
                    start=True,
                    stop=True,
                )
        # tot = mlp + c; split into fp16-exact hi + residual; interleave [hi|lo] pairs
        tot = work.tile([128, 16], dt.float32, tag="tot")
        nc.vector.tensor_tensor(tot[:], mlp_ps[:], c_all[:, s * 16 : (s + 1) * 16], Alu.add)
        tothi = work.tile([128, 16], dt.int32, tag="tothi")
        nc.vector.tensor_scalar(tothi[:], tot[:].bitcast(dt.int32), TRUNC_MASK, None, Alu.bitwise_and)
        totmov = work.tile([128, 32], dt.float16, tag="totmov")
        tm = totmov[:].rearrange("p (t two) -> p t two", two=2)
        nc.vector.tensor_copy(tm[:, :, 0], tothi[:].bitcast(dt.float32))
        nc.vector.tensor_tensor(tm[:, :, 1], tot[:], tothi[:].bitcast(dt.float32), Alu.subtract)
        for k in range(16):
            t = s * 16 + k
            nc.tensor.matmul(
                e_ps[0:gpt, 2 * t : 2 * t + 2],
                st[:, k * gpt : (k + 1) * gpt],
                totmov[:, 2 * k : 2 * k + 2],
                start=True,
                stop=True,
            )

    e_sb = const.tile([gpt, 2 * T], dt.float32)
    nc.vector.tensor_copy(e_sb[:], e_ps[0:gpt, :])
    nc.sync.dma_start(e_out, e_sb[:])


def _build_program(T, gpt=GPT, cpath_chunks=4, silu_mode="act"):
    A_ = 128 * T
    nc = bacc.Bacc("TRN2", target_bir_lowering=False, debug=False)
    shapes = {
        "xt1": ([128, A_], dt.bfloat16),
        "xt2": ([64, A_], dt.bfloat16),
        "seg": ([128, T * gpt], dt.float16),
        "nat": ([128, T * NS], dt.float32),
        "wa": ([128, 128], dt.bfloat16),
        "wb": ([64, 128], dt.bfloat16),
        "w2c": ([128, 1], dt.float16),
        "beff": ([128, 1], dt.float32),
        "shiftsb": ([128, NS], dt.float32),
        "wpow": ([128, NS], dt.float32),
        "iota10n": ([128, NS], dt.float32),
    }
    ins = {name: nc.declare_dram_parameter(name, list(sh), d, isOutput=False).ap() for name, (sh, d) in shapes.items()}
    e_out = nc.declare_dram_parameter("e_out", [gpt, 2 * T], dt.float32, isOutput=True).ap()
    with tile.TileContext(nc) as tc:
        _emit_body(tc, T, ins, e_out, gpt=gpt, cpath_chunks=cpath_chunks, silu_mode=silu_mode)
    nc.finalize()
    return nc


def _stage_params(pca_mean, pca_components, W1, b1, W2, b2, shifts):
    W_eff = (W1.astype(np.float64) @ pca_components.astype(np.float64)).T  # [192, 128]
    b_eff = b1.astype(np.float64) - W_eff.T @ pca_mean.astype(np.float64)
    W_eff = W_eff.astype(np.float32)
    bf = ml_dtypes.bfloat16
    return {
        "wa": np.ascontiguousarray(W_eff[:128]).astype(bf),
        "wb": np.ascontiguousarray(W_eff[128:]).astype(bf),
        "w2c": np.ascontiguousarray(W2.reshape(128, 1)).astype(np.float16),
        "beff": b_eff.astype(np.float32).reshape(128, 1),
        "shiftsb": np.broadcast_to((shifts + b2[0]).astype(np.float32), (128, NS)).copy(),
        "wpow": np.broadcast_to((2.0 ** (9 - np.arange(NS))).astype(np.float32), (128, NS)).copy(),
        "iota10n": np.broadcast_to((-np.arange(NS)).astype(np.float32), (128, NS)).copy(),
    }


def _stage_core_inputs(x_c, na_c, bm_c, gpt=GPT):
    """Sort one core's atoms by graph, pad to A, build device arrays + merge map."""
    n = x_c.shape[0]
    bf = ml_dtypes.bfloat16
    perm = np.argsort(bm_c, kind="stable")
    bm_s = bm_c[perm]

    xt = np.zeros((192, A), dtype=bf)
    xt[:, :n] = x_c[perm].T.astype(bf)
    nat = np.zeros((A, NS), dtype=np.float32)
    nat[:n] = na_c[perm]
    nat = np.ascontiguousarray(nat.reshape(T, 128, NS).transpose(1, 0, 2).reshape(128, T * NS))

    # segment matrices: new-graph flags / local ranks within each tile
    a_idx = np.arange(n)
    f = np.empty(n, dtype=bool)
    f[0] = True
    f[1:] = bm_s[1:] != bm_s[:-1]
    f |= a_idx % 128 == 0
    tile_of = a_idx // 128
    seg_start_rank = np.cumsum(f) - 1
    first_in_tile = np.searchsorted(tile_of, np.arange(T), side="left")
    # rank within tile = cumulative new-graph count since tile start
    base = seg_start_rank[np.minimum(first_in_tile, n - 1)]
    rank = seg_start_rank - base[tile_of]
    if n:
        assert rank.max() < gpt, f"graphs per tile exceeded {gpt}: {rank.max() + 1}"
    seg = np.zeros((T, 128, gpt), dtype=np.float16)
    seg[tile_of, a_idx % 128, rank] = 1.0
    seg = np.ascontiguousarray(seg.transpose(1, 0, 2).reshape(128, T * gpt))

    merge_tile = tile_of[f[:n]]
    merge_rank = rank[f[:n]]
    merge_graph = bm_s[f[:n]]
    return (
        {
            "xt1": np.ascontiguousarray(xt[:128]),
            "xt2": np.ascontiguousarray(xt[128:]),
            "seg": seg,
            "nat": nat,
        },
        (merge_tile.astype(np.int64), merge_rank.astype(np.int64), merge_graph.astype(np.int64)),
    )


def _get_program(gpt):
    key = (T, gpt, "act")
    if key not in _PROGRAM_CACHE:
        _PROGRAM_CACHE[key] = _build_program(T, gpt=gpt, silu_mode="act")
    return _PROGRAM_CACHE[key]


def _max_graphs_per_tile(bm_c):
    bm_s = np.sort(bm_c)
    n = len(bm_s)
    if n == 0:
        return 1
    f = np.empty(n, dtype=bool)
    f[0] = True
    f[1:] = bm_s[1:] != bm_s[:-1]
    f |= np.arange(n) % 128 == 0
    ranks = np.cumsum(f) - 1
    starts = ranks[np.arange(0, n, 128)]
    counts = np.diff(np.append(starts, ranks[-1] + 1))
    return int(counts.max())


def kernel(x, node_attrs, batch_map, base_energy, pca_mean, pca_components, W1, b1, W2, b2, shifts, _trace=False):
    x = np.asarray(x, dtype=np.float32)
    node_attrs = np.asarray(node_attrs, dtype=np.float32)
    batch_map = np.asarray(batch_map).astype(np.int64)
    base_energy = np.asarray(base_energy, dtype=np.float32)
    params = _stage_params(
        np.asarray(pca_mean, np.float32),
        np.asarray(pca_components, np.float32),
        np.asarray(W1, np.float32),
        np.asarray(b1, np.float32),
        np.asarray(W2, np.float32),
        np.asarray(b2, np.float32),
        np.asarray(shifts, np.float32),
    )

    n = x.shape[0]
    bounds = [min((n + N_CORES - 1) // N_CORES * c, n) for c in range(N_CORES + 1)]
    need = max(_max_graphs_per_tile(batch_map[bounds[c] : bounds[c + 1]]) for c in range(N_CORES))
    gpt = next(g for g in (32, 64, 128) if g >= need)
    in_maps, merges = [], []
    for c in range(N_CORES):
        s, e = bounds[c], bounds[c + 1]
        m, mg = _stage_core_inputs(x[s:e], node_attrs[s:e], batch_map[s:e], gpt=gpt)
        m.update(params)
        in_maps.append(m)
        merges.append(mg)

    nc = _get_program(gpt)
    res = run_bass_kernel_spmd(nc, in_maps, list(range(N_CORES)), trace=_trace)
    delta = np.zeros(N_GRAPHS, dtype=np.float64)
    for c in range(N_CORES):
        e_dev = np.asarray(res.results[c]["e_out"], dtype=np.float64)  # [GPT, 2T]
        mt, mr, mg = merges[c]
        vals = e_dev[mr, 2 * mt] + e_dev[mr, 2 * mt + 1]
        np.add.at(delta, mg, vals)
    delta = delta.astype(np.float32)
    final = base_energy + delta
    if _trace:
        kernel._last_result = res
    return final, delta


# revision 11
# speedup vs baseline: 5.1216x; 1.0613x over previous
"""Trainium2 kernel for nn_CachedReadoutModel (PCA -> MLP -> species shift -> segment sum).

Strategy (8 NeuronCores, data-parallel over atoms):
  host:  fold PCA into layer 1 (W_eff = (W1 @ pca_components).T, b_eff = b1 - W_eff.T mean);
         slice 1M atoms into 8 ranges; within each range STABLE-SORT atoms by
         batch_map so each 128-atom tile spans <= 32 consecutive graphs; stage x
         transposed (feature-major) in bf16; precompute per-tile segment matrices
         S[t] in [128 atoms, 32 local graphs] (0/1, fp16) from the sorted batch_map.
  core:  h = silu(W_eff.T x + b_eff); mlp = h . w2            (bf16/fp16 matmuls)
         tot = mlp + (shifts + b2)[argmax(node_attrs)]         (exact argmax on DVE)
         split tot = tot_hi + tot_lo (fp16-exact pieces)
         per tile: out[0:32, 2t:2t+2] = S[t]^T @ [tot_hi | tot_lo]   (PE, N=2)
  host:  scatter-add the per-tile per-local-graph partial sums into delta[16384]
         (<= 18k values per core), sum cores, final = base_energy + delta.
"""

import os
import sys

for _p in ("/opt/trn_rl_repo", "/root/.axon_site/_ro/trn_rl_repo"):
    if os.path.isdir(_p) and _p not in sys.path:
        sys.path.insert(0, _p)

from contextlib import ExitStack

import numpy as np
import ml_dtypes

import concourse.bass as bass
import concourse.tile as tile
from concourse import bacc, mybir
from concourse._compat import with_exitstack
from concourse.bass_utils import run_bass_kernel_spmd

dt = mybir.dt
Alu = mybir.AluOpType
Act = mybir.ActivationFunctionType

N_ATOMS = 1_000_000
N_GRAPHS = 16384
NS = 10
N_CORES = 8
T = 992  # tiles of 128 atoms per core; A = 126976 >= ceil(1e6/8)
A = 128 * T
GPT = 32  # default max graphs per 128-atom tile (sorted); host adapts via _pick_gpt
TRUNC_MASK = int(~np.int32(0x1FFF))  # keep 10 explicit mantissa bits -> fp16-exact

_PROGRAM_CACHE = {}


@with_exitstack
def _emit_body(ctx: ExitStack, tc, T, ins, e_out, gpt=GPT, cpath_chunks=4, silu_mode="act"):
    nc = tc.nc
    assert T % 16 == 0
    n_super = T // 16

    const = ctx.enter_context(tc.tile_pool(name="const", bufs=1))
    cpath = ctx.enter_context(tc.tile_pool(name="cpath", bufs=2))
    xpool = ctx.enter_context(tc.tile_pool(name="xpool", bufs=3))
    spool = ctx.enter_context(tc.tile_pool(name="spool", bufs=3))
    work = ctx.enter_context(tc.tile_pool(name="work", bufs=3))
    hps = ctx.enter_context(tc.tile_pool(name="hps", bufs=2, space="PSUM"))
    eps = ctx.enter_context(tc.tile_pool(name="eps", bufs=1, space="PSUM"))

    def load_const(name, shape, dtype):
        t = const.tile(shape, dtype, tag=name)
        nc.sync.dma_start(t[:], ins[name])
        return t

    wa = load_const("wa", [128, 128], dt.bfloat16)
    wb = load_const("wb", [64, 128], dt.bfloat16)
    w2c = load_const("w2c", [128, 1], dt.float16)
    beff = load_const("beff", [128, 1], dt.float32)
    shiftsb = load_const("shiftsb", [128, NS], dt.float32)
    wpow = load_const("wpow", [128, NS], dt.float32)
    iota10n = load_const("iota10n", [128, NS], dt.float32)

    # --- c table: c[p, t] = (shifts + b2)[argmax_s na[p, t, :]] (exact first-index) ---
    c_all = const.tile([128, T], dt.float32)
    assert T % cpath_chunks == 0
    Tc = T // cpath_chunks
    for ci in range(cpath_chunks):
        nat_c = cpath.tile([128, Tc * NS], dt.float32, tag="natc")
        nc.sync.dma_start(nat_c[:], ins["nat"][:, ci * Tc * NS : (ci + 1) * Tc * NS])
        nat3 = nat_c[:].rearrange("p (t s) -> p t s", s=NS)
        mx = cpath.tile([128, Tc], dt.float32, tag="mx")
        nc.vector.tensor_reduce(out=mx[:], in_=nat3, op=Alu.max, axis=mybir.AxisListType.X)
        eq = cpath.tile([128, Tc * NS], dt.float32, tag="eq")
        eq3 = eq[:].rearrange("p (t s) -> p t s", s=NS)
        nc.vector.tensor_tensor(eq3, nat3, mx[:].unsqueeze(-1).broadcast_to([128, Tc, NS]), Alu.is_equal)
        rw = cpath.tile([128, Tc * NS], dt.float32, tag="rw")
        rw3 = rw[:].rearrange("p (t s) -> p t s", s=NS)
        nc.vector.tensor_tensor(rw3, eq3, wpow[:].unsqueeze(1).broadcast_to([128, Tc, NS]), Alu.mult)
        r = cpath.tile([128, Tc], dt.float32, tag="r")
        nc.vector.tensor_reduce(out=r[:], in_=rw3, op=Alu.add, axis=mybir.AxisListType.X)
        em_i = cpath.tile([128, Tc], dt.int32, tag="emi")
        nc.vector.tensor_scalar(em_i[:], r[:].bitcast(dt.int32), 23, None, Alu.logical_shift_right)
        em = cpath.tile([128, Tc], dt.float32, tag="em")
        nc.vector.tensor_scalar(em[:], em_i[:], 136, None, Alu.subtract)
        eq2 = cpath.tile([128, Tc * NS], dt.float32, tag="eq2")
        eq23 = eq2[:].rearrange("p (t s) -> p t s", s=NS)
        nc.vector.tensor_tensor(
            eq23,
            iota10n[:].unsqueeze(1).broadcast_to([128, Tc, NS]),
            em[:].unsqueeze(-1).broadcast_to([128, Tc, NS]),
            Alu.is_equal,
        )
        cw = cpath.tile([128, Tc * NS], dt.float32, tag="cw")
        cw3 = cw[:].rearrange("p (t s) -> p t s", s=NS)
        nc.vector.tensor_tensor(cw3, eq23, shiftsb[:].unsqueeze(1).broadcast_to([128, Tc, NS]), Alu.mult)
        nc.vector.tensor_reduce(out=c_all[:, ci * Tc : (ci + 1) * Tc], in_=cw3, op=Alu.add, axis=mybir.AxisListType.X)

    # --- main loop over superblocks of 2048 atoms (16 tiles) ---
    # one 4-bank PSUM tile: cols [0, 2T) = per-tile segment sums, cols [2T, 2T+32) = two mlp slots
    assert 2 * T + 32 <= 2048
    psum_all = eps.tile([128, 2048], dt.float32)
    e_ps = psum_all[:, 0 : 2 * T]
    for s in range(n_super):
        a0 = s * 2048
        x1 = xpool.tile([128, 2048], dt.bfloat16, tag="x1")
        nc.sync.dma_start(x1[:], ins["xt1"][:, a0 : a0 + 2048])
        x2 = xpool.tile([64, 2048], dt.bfloat16, tag="x2")
        nc.sync.dma_start(x2[:], ins["xt2"][:, a0 : a0 + 2048])
        st = spool.tile([128, 16 * gpt], dt.float16, tag="st")
        nc.sync.dma_start(st[:], ins["seg"][:, s * 16 * gpt : (s + 1) * 16 * gpt])
        mlp_ps = psum_all[:, 2 * T + 16 * (s % 2) : 2 * T + 16 * (s % 2) + 16]
        for half in range(2):
            h_ps = hps.tile([128, 1024], dt.float32)
            for q in range(2):
                sl = slice((2 * half + q) * 512, (2 * half + q + 1) * 512)
                nc.tensor.matmul(h_ps[:, q * 512 : (q + 1) * 512], wa[:], x1[:, sl], start=True, stop=False)
                nc.tensor.matmul(h_ps[:, q * 512 : (q + 1) * 512], wb[:], x2[:, sl], start=False, stop=True)
            silu = work.tile([128, 1024], dt.float16, tag="silu")
            if silu_mode == "act":
                nc.scalar.activation(silu[:], h_ps[:], Act.Silu, bias=beff[:], scale=1.0)
            else:
                sg = work.tile([128, 1024], dt.float32, tag="sg")
                nc.scalar.activation(sg[:], h_ps[:], Act.Sigmoid, bias=beff[:], scale=1.0)
                hb = work.tile([128, 1024], dt.float32, tag="hb")
                nc.scalar.activation(hb[:], h_ps[:], Act.Identity, bias=beff[:], scale=1.0)
                nc.vector.tensor_tensor(silu[:], hb[:], sg[:], Alu.mult)
            for j in range(8):
                nc.tensor.matmul(
                    mlp_ps[:, half * 8 + j : half * 8 + j + 1],
                    silu[:, j * 128 : (j + 1) * 128],
                    w2c[:],
                    start=True,
                    stop=True,
                )
        # tot = mlp + c; split into fp16-exact hi + residual; interleave [hi|lo] pairs
        tot = work.tile([128, 16], dt.float32, tag="tot")
        nc.vector.tensor_tensor(tot[:], mlp_ps[:], c_all[:, s * 16 : (s + 1) * 16], Alu.add)
        tothi = work.tile([128, 16], dt.int32, tag="tothi")
        nc.vector.tensor_scalar(tothi[:], tot[:].bitcast(dt.int32), TRUNC_MASK, None, Alu.bitwise_and)
        totmov = work.tile([128, 32], dt.float16, tag="totmov")
        tm = totmov[:].rearrange("p (t two) -> p t two", two=2)
        nc.vector.tensor_copy(tm[:, :, 0], tothi[:].bitcast(dt.float32))
        nc.vector.tensor_tensor(tm[:, :, 1], tot[:], tothi[:].bitcast(dt.float32), Alu.subtract)
        for k in range(16):
            t = s * 16 + k
            nc.tensor.matmul(
                e_ps[0:gpt, 2 * t : 2 * t + 2],
                st[:, k * gpt : (k + 1) * gpt],
                totmov[:, 2 * k : 2 * k + 2],
                start=True,
                stop=True,
            )

    e_sb = const.tile([gpt, 2 * T], dt.float32)
    nc.vector.tensor_copy(e_sb[:], e_ps[0:gpt, :])
    nc.sync.dma_start(e_out, e_sb[:])


def _build_program(T, gpt=GPT, cpath_chunks=4, silu_mode="act"):
    A_ = 128 * T
    nc = bacc.Bacc("TRN2", target_bir_lowering=False, debug=False)
    shapes = {
        "xt1": ([128, A_], dt.bfloat16),
        "xt2": ([64, A_], dt.bfloat16),
        "seg": ([128, T * gpt], dt.float16),
        "nat": ([128, T * NS], dt.float32),
        "wa": ([128, 128], dt.bfloat16),
        "wb": ([64, 128], dt.bfloat16),
        "w2c": ([128, 1], dt.float16),
        "beff": ([128, 1], dt.float32),
        "shiftsb": ([128, NS], dt.float32),
        "wpow": ([128, NS], dt.float32),
        "iota10n": ([128, NS], dt.float32),
    }
    ins = {name: nc.declare_dram_parameter(name, list(sh), d, isOutput=False).ap() for name, (sh, d) in shapes.items()}
    e_out = nc.declare_dram_parameter("e_out", [gpt, 2 * T], dt.float32, isOutput=True).ap()
    with tile.TileContext(nc) as tc:
        _emit_body(tc, T, ins, e_out, gpt=gpt, cpath_chunks=cpath_chunks, silu_mode=silu_mode)
    nc.finalize()
    return nc


def _stage_params(pca_mean, pca_components, W1, b1, W2, b2, shifts):
    W_eff = (W1.astype(np.float64) @ pca_components.astype(np.float64)).T  # [192, 128]
    b_eff = b1.astype(np.float64) - W_eff.T @ pca_mean.astype(np.float64)
    W_eff = W_eff.astype(np.float32)
    bf = ml_dtypes.bfloat16
    return {
        "wa": np.ascontiguousarray(W_eff[:128]).astype(bf),
        "wb": np.ascontiguousarray(W_eff[128:]).astype(bf),
        "w2c": np.ascontiguousarray(W2.reshape(128, 1)).astype(np.float16),
        "beff": b_eff.astype(np.float32).reshape(128, 1),
        "shiftsb": np.broadcast_to((shifts + b2[0]).astype(np.float32), (128, NS)).copy(),
        "wpow": np.broadcast_to((2.0 ** (9 - np.arange(NS))).astype(np.float32), (128, NS)).copy(),
        "iota10n": np.broadcast_to((-np.arange(NS)).astype(np.float32), (128, NS)).copy(),
    }


def _stage_core_inputs(x_c, na_c, bm_c, gpt=GPT):
    """Sort one core's atoms by graph, pad to A, build device arrays + merge map."""
    n = x_c.shape[0]
    bf = ml_dtypes.bfloat16
    perm = np.argsort(bm_c, kind="stable")
    bm_s = bm_c[perm]

    xt = np.zeros((192, A), dtype=bf)
    xt[:, :n] = x_c[perm].T.astype(bf)
    nat = np.zeros((A, NS), dtype=np.float32)
    nat[:n] = na_c[perm]
    nat = np.ascontiguousarray(nat.reshape(T, 128, NS).transpose(1, 0, 2).reshape(128, T * NS))

    # segment matrices: new-graph flags / local ranks within each tile
    a_idx = np.arange(n)
    f = np.empty(n, dtype=bool)
    f[0] = True
    f[1:] = bm_s[1:] != bm_s[:-1]
    f |= a_idx % 128 == 0
    tile_of = a_idx // 128
    seg_start_rank = np.cumsum(f) - 1
    first_in_tile = np.searchsorted(tile_of, np.arange(T), side="left")
    # rank within tile = cumulative new-graph count since tile start
    base = seg_start_rank[np.minimum(first_in_tile, n - 1)]
    rank = seg_start_rank - base[tile_of]
    if n:
        assert rank.max() < gpt, f"graphs per tile exceeded {gpt}: {rank.max() + 1}"
    seg = np.zeros((T, 128, gpt), dtype=np.float16)
    seg[tile_of, a_idx % 128, rank] = 1.0
    seg = np.ascontiguousarray(seg.transpose(1, 0, 2).reshape(128, T * gpt))

    merge_tile = tile_of[f[:n]]
    merge_rank = rank[f[:n]]
    merge_graph = bm_s[f[:n]]
    return (
        {
            "xt1": np.ascontiguousarray(xt[:128]),
            "xt2": np.ascontiguousarray(xt[128:]),
            "seg": seg,
            "nat": nat,
        },
        (merge_tile.astype(np.int64), merge_rank.astype(np.int64), merge_graph.astype(np.int64)),
    )


def _get_program(gpt):
    key = (T, gpt, "act")
    if key not in _PROGRAM_CACHE:
        _PROGRAM_CACHE[key] = _build_program(T, gpt=gpt, silu_mode="act")
    return _PROGRAM_CACHE[key]


def _max_graphs_per_tile(bm_c):
    bm_s = np.sort(bm_c)
    n = len(bm_s)
    if n == 0:
        return 1
    f = np.empty(n, dtype=bool)
    f[0] = True
    f[1:] = bm_s[1:] != bm_s[:-1]
    f |= np.arange(n) % 128 == 0
    ranks = np.cumsum(f) - 1
    starts = ranks[np.arange(0, n, 128)]
    counts = np.diff(np.append(starts, ranks[-1] + 1))
    return int(counts.max())


def kernel(x, node_attrs, batch_map, base_energy, pca_mean, pca_components, W1, b1, W2, b2, shifts, _trace=False):
    x = np.asarray(x, dtype=np.float32)
    node_attrs = np.asarray(node_attrs, dtype=np.float32)
    batch_map = np.asarray(batch_map).astype(np.int64)
    base_energy = np.asarray(base_energy, dtype=np.float32)
    params = _stage_params(
        np.asarray(pca_mean, np.float32),
        np.asarray(pca_components, np.float32),
        np.asarray(W1, np.float32),
        np.asarray(b1, np.float32),
        np.asarray(W2, np.float32),
        np.asarray(b2, np.float32),
        np.asarray(shifts, np.float32),
    )

    n = x.shape[0]
    bounds = [min((n + N_CORES - 1) // N_CORES * c, n) for c in range(N_CORES + 1)]
    need = max(_max_graphs_per_tile(batch_map[bounds[c] : bounds[c + 1]]) for c in range(N_CORES))
    gpt = next(g for g in (32, 64, 128) if g >= need)
    in_maps, merges = [], []
    for c in range(N_CORES):
        s, e = bounds[c], bounds[c + 1]
        m, mg = _stage_core_inputs(x[s:e], node_attrs[s:e], batch_map[s:e], gpt=gpt)
        m.update(params)
        in_maps.append(m)
        merges.append(mg)

    nc = _get_program(gpt)
    res = run_bass_kernel_spmd(nc, in_maps, list(range(N_CORES)), trace=_trace)
    delta = np.zeros(N_GRAPHS, dtype=np.float64)
    for c in range(N_CORES):
        e_dev = np.asarray(res.results[c]["e_out"], dtype=np.float64)  # [gpt, 2T]
        mt, mr, mg = merges[c]
        vals = e_dev[mr, 2 * mt] + e_dev[mr, 2 * mt + 1]
        np.add.at(delta, mg, vals)
    delta = delta.astype(np.float32)
    final = base_energy + delta
    if _trace:
        kernel._last_result = res
    return final, delta


# revision 13
# speedup vs baseline: 5.8371x; 1.1397x over previous
"""Trainium2 kernel for nn_CachedReadoutModel (PCA -> MLP -> species shift -> segment sum).

Strategy (8 NeuronCores, data-parallel over atoms):
  host:  fold PCA into layer 1 (W_eff = (W1 @ pca_components).T, b_eff = b1 - W_eff.T mean);
         slice 1M atoms into 8 ranges; within each range STABLE-SORT atoms by
         batch_map so each 128-atom tile spans <= 32 consecutive graphs; stage x
         transposed (feature-major) in bf16; precompute per-tile segment matrices
         S[t] in [128 atoms, 32 local graphs] (0/1, fp16) from the sorted batch_map.
  core:  h = silu(W_eff.T x + b_eff); mlp = h . w2            (bf16/fp16 matmuls)
         tot = mlp + (shifts + b2)[argmax(node_attrs)]         (exact argmax on DVE)
         split tot = tot_hi + tot_lo (fp16-exact pieces)
         per tile: out[0:32, 2t:2t+2] = S[t]^T @ [tot_hi | tot_lo]   (PE, N=2)
  host:  scatter-add the per-tile per-local-graph partial sums into delta[16384]
         (<= 18k values per core), sum cores, final = base_energy + delta.
"""

import os
import sys

for _p in ("/opt/trn_rl_repo", "/root/.axon_site/_ro/trn_rl_repo"):
    if os.path.isdir(_p) and _p not in sys.path:
        sys.path.insert(0, _p)

from contextlib import ExitStack

import numpy as np
import ml_dtypes

import concourse.bass as bass
import concourse.tile as tile
from concourse import bacc, mybir
from concourse._compat import with_exitstack
from concourse.bass_utils import run_bass_kernel_spmd

dt = mybir.dt
Alu = mybir.AluOpType
Act = mybir.ActivationFunctionType

N_ATOMS = 1_000_000
N_GRAPHS = 16384
NS = 10
N_CORES = 8
T = 992  # tiles of 128 atoms per core; A = 126976 >= ceil(1e6/8)
A = 128 * T
GPT = 32  # default max graphs per 128-atom tile (sorted); host adapts via _pick_gpt
TRUNC_MASK = int(~np.int32(0x1FFF))  # keep 10 explicit mantissa bits -> fp16-exact

_PROGRAM_CACHE = {}


@with_exitstack
def _emit_body(ctx: ExitStack, tc, T, ins, e_out, gpt=GPT, cpath_chunks=16, silu_mode="act"):
    nc = tc.nc
    assert T % 16 == 0
    n_super = T // 16

    const = ctx.enter_context(tc.tile_pool(name="const", bufs=1))
    cpath = ctx.enter_context(tc.tile_pool(name="cpath", bufs=2))
    xpool = ctx.enter_context(tc.tile_pool(name="xpool", bufs=3))
    spool = ctx.enter_context(tc.tile_pool(name="spool", bufs=3))
    work = ctx.enter_context(tc.tile_pool(name="work", bufs=3))
    hps = ctx.enter_context(tc.tile_pool(name="hps", bufs=2, space="PSUM"))
    eps = ctx.enter_context(tc.tile_pool(name="eps", bufs=1, space="PSUM"))

    def load_const(name, shape, dtype):
        t = const.tile(shape, dtype, tag=name)
        nc.sync.dma_start(t[:], ins[name])
        return t

    wa = load_const("wa", [128, 128], dt.bfloat16)
    wb = load_const("wb", [64, 128], dt.bfloat16)
    w2c = load_const("w2c", [128, 1], dt.float16)
    beff = load_const("beff", [128, 1], dt.float32)
    shiftsb = load_const("shiftsb", [128, NS], dt.float32)
    wpow = load_const("wpow", [128, NS], dt.float32)
    iota10n = load_const("iota10n", [128, NS], dt.float32)

    # --- c table: c[p, t] = (shifts + b2)[argmax_s na[p, t, :]] (exact first-index) ---
    # emitted in chunks interleaved with the main loop so DVE work overlaps PE work
    c_all = const.tile([128, T], dt.float32)
    assert T % cpath_chunks == 0
    Tc = T // cpath_chunks

    def emit_cpath_chunk(ci):
        nat_c = cpath.tile([128, Tc * NS], dt.float32, tag="natc")
        nc.sync.dma_start(nat_c[:], ins["nat"][:, ci * Tc * NS : (ci + 1) * Tc * NS])
        nat3 = nat_c[:].rearrange("p (t s) -> p t s", s=NS)
        mx = cpath.tile([128, Tc], dt.float32, tag="mx")
        nc.vector.tensor_reduce(out=mx[:], in_=nat3, op=Alu.max, axis=mybir.AxisListType.X)
        eq = cpath.tile([128, Tc * NS], dt.float32, tag="eq")
        eq3 = eq[:].rearrange("p (t s) -> p t s", s=NS)
        nc.vector.tensor_tensor(eq3, nat3, mx[:].unsqueeze(-1).broadcast_to([128, Tc, NS]), Alu.is_equal)
        rw = cpath.tile([128, Tc * NS], dt.float32, tag="rw")
        rw3 = rw[:].rearrange("p (t s) -> p t s", s=NS)
        nc.vector.tensor_tensor(rw3, eq3, wpow[:].unsqueeze(1).broadcast_to([128, Tc, NS]), Alu.mult)
        r = cpath.tile([128, Tc], dt.float32, tag="r")
        nc.vector.tensor_reduce(out=r[:], in_=rw3, op=Alu.add, axis=mybir.AxisListType.X)
        em_i = cpath.tile([128, Tc], dt.int32, tag="emi")
        nc.vector.tensor_scalar(em_i[:], r[:].bitcast(dt.int32), 23, None, Alu.logical_shift_right)
        em = cpath.tile([128, Tc], dt.float32, tag="em")
        nc.vector.tensor_scalar(em[:], em_i[:], 136, None, Alu.subtract)
        eq2 = cpath.tile([128, Tc * NS], dt.float32, tag="eq2")
        eq23 = eq2[:].rearrange("p (t s) -> p t s", s=NS)
        nc.vector.tensor_tensor(
            eq23,
            iota10n[:].unsqueeze(1).broadcast_to([128, Tc, NS]),
            em[:].unsqueeze(-1).broadcast_to([128, Tc, NS]),
            Alu.is_equal,
        )
        cw = cpath.tile([128, Tc * NS], dt.float32, tag="cw")
        cw3 = cw[:].rearrange("p (t s) -> p t s", s=NS)
        nc.vector.tensor_tensor(cw3, eq23, shiftsb[:].unsqueeze(1).broadcast_to([128, Tc, NS]), Alu.mult)
        nc.vector.tensor_reduce(out=c_all[:, ci * Tc : (ci + 1) * Tc], in_=cw3, op=Alu.add, axis=mybir.AxisListType.X)

    # --- main loop over superblocks of 2048 atoms (16 tiles) ---
    # one 4-bank PSUM tile: cols [0, 2T) = per-tile segment sums, cols [2T, 2T+32) = two mlp slots
    assert 2 * T + 32 <= 2048
    psum_all = eps.tile([128, 2048], dt.float32)
    e_ps = psum_all[:, 0 : 2 * T]
    next_chunk = 0
    for s in range(n_super):
        while next_chunk < cpath_chunks and s >= (next_chunk * n_super) // cpath_chunks - 2:
            emit_cpath_chunk(next_chunk)
            next_chunk += 1
        a0 = s * 2048
        x1 = xpool.tile([128, 2048], dt.bfloat16, tag="x1")
        nc.sync.dma_start(x1[:], ins["xt1"][:, a0 : a0 + 2048])
        x2 = xpool.tile([64, 2048], dt.bfloat16, tag="x2")
        nc.sync.dma_start(x2[:], ins["xt2"][:, a0 : a0 + 2048])
        if s == 0 and gpt <= 64:
            # HAM warm-up: ~5us of dense array work flips the PE clock gate to
            # 2.4 GHz before the real stream starts; scratch output lands in
            # psum rows 64..127 of the segment area, which the host never reads.
            for w in range(10):
                nc.tensor.matmul(psum_all[64:128, 0:512], wa[:, 0:64], x1[:, 0:512], start=True, stop=True)
        st = spool.tile([128, 16 * gpt], dt.float16, tag="st")
        nc.sync.dma_start(st[:], ins["seg"][:, s * 16 * gpt : (s + 1) * 16 * gpt])
        mlp_ps = psum_all[:, 2 * T + 16 * (s % 2) : 2 * T + 16 * (s % 2) + 16]
        for half in range(2):
            h_ps = hps.tile([128, 1024], dt.float32)
            for q in range(2):
                sl = slice((2 * half + q) * 512, (2 * half + q + 1) * 512)
                nc.tensor.matmul(h_ps[:, q * 512 : (q + 1) * 512], wa[:], x1[:, sl], start=True, stop=False)
                nc.tensor.matmul(h_ps[:, q * 512 : (q + 1) * 512], wb[:], x2[:, sl], start=False, stop=True)
            silu = work.tile([128, 1024], dt.float16, tag="silu")
            if silu_mode == "act":
                nc.scalar.activation(silu[:], h_ps[:], Act.Silu, bias=beff[:], scale=1.0)
            else:
                sg = work.tile([128, 1024], dt.float32, tag="sg")
                nc.scalar.activation(sg[:], h_ps[:], Act.Sigmoid, bias=beff[:], scale=1.0)
                hb = work.tile([128, 1024], dt.float32, tag="hb")
                nc.scalar.activation(hb[:], h_ps[:], Act.Identity, bias=beff[:], scale=1.0)
                nc.vector.tensor_tensor(silu[:], hb[:], sg[:], Alu.mult)
            for j in range(8):
                nc.tensor.matmul(
                    mlp_ps[:, half * 8 + j : half * 8 + j + 1],
                    silu[:, j * 128 : (j + 1) * 128],
                    w2c[:],
                    start=True,
                    stop=True,
                )
        # tot = mlp + c; split into fp16-exact hi + residual; interleave [hi|lo] pairs
        tot = work.tile([128, 16], dt.float32, tag="tot")
        nc.vector.tensor_tensor(tot[:], mlp_ps[:], c_all[:, s * 16 : (s + 1) * 16], Alu.add)
        tothi = work.tile([128, 16], dt.int32, tag="tothi")
        nc.vector.tensor_scalar(tothi[:], tot[:].bitcast(dt.int32), TRUNC_MASK, None, Alu.bitwise_and)
        totmov = work.tile([128, 32], dt.float16, tag="totmov")
        tm = totmov[:].rearrange("p (t two) -> p t two", two=2)
        nc.vector.tensor_copy(tm[:, :, 0], tothi[:].bitcast(dt.float32))
        nc.vector.tensor_tensor(tm[:, :, 1], tot[:], tothi[:].bitcast(dt.float32), Alu.subtract)
        for k in range(16):
            t = s * 16 + k
            nc.tensor.matmul(
                e_ps[0:gpt, 2 * t : 2 * t + 2],
                st[:, k * gpt : (k + 1) * gpt],
                totmov[:, 2 * k : 2 * k + 2],
                start=True,
                stop=True,
            )

    e_sb = const.tile([gpt, 2 * T], dt.float32)
    nc.vector.tensor_copy(e_sb[:], e_ps[0:gpt, :])
    nc.sync.dma_start(e_out, e_sb[:])


def _build_program(T, gpt=GPT, cpath_chunks=16, silu_mode="act"):
    A_ = 128 * T
    nc = bacc.Bacc("TRN2", target_bir_lowering=False, debug=False)
    shapes = {
        "xt1": ([128, A_], dt.bfloat16),
        "xt2": ([64, A_], dt.bfloat16),
        "seg": ([128, T * gpt], dt.float16),
        "nat": ([128, T * NS], dt.float32),
        "wa": ([128, 128], dt.bfloat16),
        "wb": ([64, 128], dt.bfloat16),
        "w2c": ([128, 1], dt.float16),
        "beff": ([128, 1], dt.float32),
        "shiftsb": ([128, NS], dt.float32),
        "wpow": ([128, NS], dt.float32),
        "iota10n": ([128, NS], dt.float32),
    }
    ins = {name: nc.declare_dram_parameter(name, list(sh), d, isOutput=False).ap() for name, (sh, d) in shapes.items()}
    e_out = nc.declare_dram_parameter("e_out", [gpt, 2 * T], dt.float32, isOutput=True).ap()
    with tile.TileContext(nc) as tc:
        _emit_body(tc, T, ins, e_out, gpt=gpt, cpath_chunks=cpath_chunks, silu_mode=silu_mode)
    nc.finalize()
    return nc


def _stage_params(pca_mean, pca_components, W1, b1, W2, b2, shifts):
    W_eff = (W1.astype(np.float64) @ pca_components.astype(np.float64)).T  # [192, 128]
    b_eff = b1.astype(np.float64) - W_eff.T @ pca_mean.astype(np.float64)
    W_eff = W_eff.astype(np.float32)
    bf = ml_dtypes.bfloat16
    return {
        "wa": np.ascontiguousarray(W_eff[:128]).astype(bf),
        "wb": np.ascontiguousarray(W_eff[128:]).astype(bf),
        "w2c": np.ascontiguousarray(W2.reshape(128, 1)).astype(np.float16),
        "beff": b_eff.astype(np.float32).reshape(128, 1),
        "shiftsb": np.broadcast_to((shifts + b2[0]).astype(np.float32), (128, NS)).copy(),
        "wpow": np.broadcast_to((2.0 ** (9 - np.arange(NS))).astype(np.float32), (128, NS)).copy(),
        "iota10n": np.broadcast_to((-np.arange(NS)).astype(np.float32), (128, NS)).copy(),
    }


def _stage_core_inputs(x_c, na_c, bm_c, gpt=GPT):
    """Sort one core's atoms by graph, pad to A, build device arrays + merge map."""
    n = x_c.shape[0]
    bf = ml_dtypes.bfloat16
    perm = np.argsort(bm_c, kind="stable")
    bm_s = bm_c[perm]

    xt = np.zeros((192, A), dtype=bf)
    xt[:, :n] = x_c[perm].T.astype(bf)
    nat = np.zeros((A, NS), dtype=np.float32)
    nat[:n] = na_c[perm]
    nat = np.ascontiguousarray(nat.reshape(T, 128, NS).transpose(1, 0, 2).reshape(128, T * NS))

    # segment matrices: new-graph flags / local ranks within each tile
    a_idx = np.arange(n)
    f = np.empty(n, dtype=bool)
    f[0] = True
    f[1:] = bm_s[1:] != bm_s[:-1]
    f |= a_idx % 128 == 0
    tile_of = a_idx // 128
    seg_start_rank = np.cumsum(f) - 1
    first_in_tile = np.searchsorted(tile_of, np.arange(T), side="left")
    # rank within tile = cumulative new-graph count since tile start
    base = seg_start_rank[np.minimum(first_in_tile, n - 1)]
    rank = seg_start_rank - base[tile_of]
    if n:
        assert rank.max() < gpt, f"graphs per tile exceeded {gpt}: {rank.max() + 1}"
    seg = np.zeros((T, 128, gpt), dtype=np.float16)
    seg[tile_of, a_idx % 128, rank] = 1.0
    seg = np.ascontiguousarray(seg.transpose(1, 0, 2).reshape(128, T * gpt))

    merge_tile = tile_of[f[:n]]
    merge_rank = rank[f[:n]]
    merge_graph = bm_s[f[:n]]
    return (
        {
            "xt1": np.ascontiguousarray(xt[:128]),
            "xt2": np.ascontiguousarray(xt[128:]),
            "seg": seg,
            "nat": nat,
        },
        (merge_tile.astype(np.int64), merge_rank.astype(np.int64), merge_graph.astype(np.int64)),
    )


def _get_program(gpt):
    key = (T, gpt, "act")
    if key not in _PROGRAM_CACHE:
        _PROGRAM_CACHE[key] = _build_program(T, gpt=gpt, silu_mode="act")
    return _PROGRAM_CACHE[key]


def _max_graphs_per_tile(bm_c):
    bm_s = np.sort(bm_c)
    n = len(bm_s)
    if n == 0:
        return 1
    f = np.empty(n, dtype=bool)
    f[0] = True
    f[1:] = bm_s[1:] != bm_s[:-1]
    f |= np.arange(n) % 128 == 0
    ranks = np.cumsum(f) - 1
    starts = ranks[np.arange(0, n, 128)]
    counts = np.diff(np.append(starts, ranks[-1] + 1))
    return int(counts.max())


def kernel(x, node_attrs, batch_map, base_energy, pca_mean, pca_components, W1, b1, W2, b2, shifts, _trace=False):
    x = np.asarray(x, dtype=np.float32)
    node_attrs = np.asarray(node_attrs, dtype=np.float32)
    batch_map = np.asarray(batch_map).astype(np.int64)
    base_energy = np.asarray(base_energy, dtype=np.float32)
    params = _stage_params(
        np.asarray(pca_mean, np.float32),
        np.asarray(pca_components, np.float32),
        np.asarray(W1, np.float32),
        np.asarray(b1, np.float32),
        np.asarray(W2, np.float32),
        np.asarray(b2, np.float32),
        np.asarray(shifts, np.float32),
    )

    n = x.shape[0]
    bounds = [min((n + N_CORES - 1) // N_CORES * c, n) for c in range(N_CORES + 1)]
    need = max(_max_graphs_per_tile(batch_map[bounds[c] : bounds[c + 1]]) for c in range(N_CORES))
    gpt = next(g for g in (32, 64, 128) if g >= need)
    in_maps, merges = [], []
    for c in range(N_CORES):
        s, e = bounds[c], bounds[c + 1]
        m, mg = _stage_core_inputs(x[s:e], node_attrs[s:e], batch_map[s:e], gpt=gpt)
        m.update(params)
        in_maps.append(m)
        merges.append(mg)

    nc = _get_program(gpt)
    res = run_bass_kernel_spmd(nc, in_maps, list(range(N_CORES)), trace=_trace)
    delta = np.zeros(N_GRAPHS, dtype=np.float64)
    for c in range(N_CORES):
        e_dev = np.asarray(res.results[c]["e_out"], dtype=np.float64)  # [gpt, 2T]
        mt, mr, mg = merges[c]
        vals = e_dev[mr, 2 * mt] + e_dev[mr, 2 * mt + 1]
        np.add.at(delta, mg, vals)
    delta = delta.astype(np.float32)
    final = base_energy + delta
    if _trace:
        kernel._last_result = res
    return final, delta
